# revision 26
# baseline (speedup 1.0000x reference)
"""DeepForest (nn_DeepForest_30356828848186) Trainium2 Bass kernel.

Strategy: data-parallel over batch across the 8 NeuronCores (B=8192 ->
1024 rows/core, forest parameters replicated) -- no collectives needed.

Per core:
  * 620 node slots (level-major: slot = (m-1)*20 + t), 310 pairs; each
    pair packs 2 nodes x 64 hidden units onto the 128 PE partitions.
  * matmul1 computes y = x @ W1dense per pair into PSUM [128, 1024],
    where W1dense is the per-node feature-scattered W1 with |W2| folded
    in (w*relu(y) = sign(w)*relu(|w|*y)).  Error-compensated scheme
    validated numerically on the exact seed-0 data (err std ~5e-7,
    0/8192 label mismatches):
      pass1: f32r  Wr  @ xr      (tf32 hi x hi)
      pass2: bf16  Wr_b @ xl_b   (x residual correction)
      pass3: bf16  Wl_b @ xr_b   (W residual correction)
  * relu+bias via ScalarE activation (per-partition bias), producing an
    f32r-rounded hr and fp32 h; hl = h - hr (f32r) on VectorE.
  * z = sum_h sign(W2)*relu(...) via sign-matrix matmuls accumulated
    per 64-pair group into PSUM [128 slots, 512 B]; z = sgn@hr + sgn@hl.
  * gate bits s = (z > -b2) via per-partition threshold compare; PE
    transposes per group (interleaved with the matmul stream), then the
    tree walk as bf16 mask algebra on DVE (2x 16-bit throughput).

Self-contained: hardcodes all shapes; only needs numpy + concourse.
"""
import sys

import numpy as np
import ml_dtypes

sys.path.insert(0, "/opt/trn_rl_repo")

import concourse.bass as bass  # noqa: E402
import concourse.bacc as bacc  # noqa: E402
import concourse.mybir as mybir  # noqa: E402
from concourse.tile import TileContext  # noqa: E402
from concourse import bass_utils  # noqa: E402
import bass_rust as _bass_rust  # noqa: E402

F32 = mybir.dt.float32
F32R = mybir.dt.float32r
BF16 = mybir.dt.bfloat16
AX = mybir.AxisListType
OP = mybir.AluOpType
ACT_RELU = mybir.ActivationFunctionType.Relu

T, M, D, B, F, H = 20, 31, 5, 8192, 256, 64
L = 16
ELIDE = True
LAG_CFG = 2
NSLOT = 620          # node slots, no padding
NP = NSLOT // 2      # 310 node pairs
NZ = 5               # z-tiles (slot groups of 128)
NCORE = 8
BLOC = B // NCORE    # 1024
NCLS = 10
NCF = NP + 8 + 128   # f32 consts cols: b1c, thr(pad 8), ident
NCB = 2560 + 2560 + 80  # bf16 consts cols: blbr8, brc8, wvec8 (x8 bt)


def _tf32(a: np.ndarray) -> np.ndarray:
    bits = np.asarray(a, np.float32).view(np.uint32) + np.uint32(0x1000)
    bits &= np.uint32(0xFFFFE000)
    return bits.view(np.float32)


def _bf16(a: np.ndarray) -> np.ndarray:
    return np.asarray(a, np.float32).astype(ml_dtypes.bfloat16)


def _pack_inputs(x, W1, b1, W2, b2, feats, best_left, best_right):
    x = np.asarray(x, np.float32)
    W1 = np.asarray(W1, np.float32)
    b1 = np.asarray(b1, np.float32)
    W2 = np.asarray(W2, np.float32)
    b2 = np.asarray(b2, np.float32)
    feats = np.asarray(feats)
    bl = np.asarray(best_left).astype(np.float32)
    br = np.asarray(best_right).astype(np.float32)

    absW2 = np.abs(W2)
    sgnW2 = np.sign(W2).astype(np.float32)

    slot_t = np.arange(NSLOT) % 20
    slot_m = np.arange(NSLOT) // 20  # node index m-1 in 0..30

    # per-slot dense folded weights [620, 256, 64]
    W1s = (W1[slot_t, slot_m] * absW2[slot_t, slot_m][:, None, :]
           ).astype(np.float32)                      # [620, 128, 64]
    fs = feats[slot_t, slot_m]                       # [620, 128]
    W1f = np.zeros((NSLOT, F, H), np.float32)
    W1f[np.arange(NSLOT)[:, None, None], fs[:, :, None],
        np.arange(H)[None, None, :]] = W1s
    Wr = _tf32(W1f)
    Wl = _tf32(W1f - Wr)
    Wr_b = _bf16(Wr)
    Wl_b = _bf16(Wl)
    b1f = (b1[slot_t, slot_m] * absW2[slot_t, slot_m]).astype(np.float32)
    sgs = sgnW2[slot_t, slot_m]                      # [620, 64]

    # wtf [NP, 128, 384] f32: war_h0 | war_h1 | sgn
    wtf = np.zeros((NP, 128, 384), np.float32)
    Wrp = Wr.reshape(NP, 2, F, H)
    wtf[:, :, 0:64] = Wrp[:, 0, 0:128]
    wtf[:, :, 64:128] = Wrp[:, 1, 0:128]
    wtf[:, :, 128:192] = Wrp[:, 0, 128:256]
    wtf[:, :, 192:256] = Wrp[:, 1, 128:256]
    u = np.arange(NP)
    ca = (2 * u) % 128
    cb = (2 * u + 1) % 128
    wtf[u[:, None], np.arange(64)[None, :], (256 + ca)[:, None]] = \
        sgs.reshape(NP, 2, 64)[:, 0]
    wtf[u[:, None], (64 + np.arange(64))[None, :], (256 + cb)[:, None]] = \
        sgs.reshape(NP, 2, 64)[:, 1]

    # wtb [NP, 128, 512] bf16: wrb_h0 | wrb_h1 | wlb_h0 | wlb_h1
    wtb = np.zeros((NP, 128, 512), ml_dtypes.bfloat16)
    Wrbp = Wr_b.reshape(NP, 2, F, H)
    Wlbp = Wl_b.reshape(NP, 2, F, H)
    wtb[:, :, 0:64] = Wrbp[:, 0, 0:128]
    wtb[:, :, 64:128] = Wrbp[:, 1, 0:128]
    wtb[:, :, 128:192] = Wrbp[:, 0, 128:256]
    wtb[:, :, 192:256] = Wrbp[:, 1, 128:256]
    wtb[:, :, 256:320] = Wlbp[:, 0, 0:128]
    wtb[:, :, 320:384] = Wlbp[:, 1, 0:128]
    wtb[:, :, 384:448] = Wlbp[:, 0, 128:256]
    wtb[:, :, 448:512] = Wlbp[:, 1, 128:256]

    # f32 consts [128, NCF]: b1c | thr | ident
    ccf = np.zeros((128, NCF), np.float32)
    ccf[0:64, 0:NP] = b1f.reshape(NP, 2, 64)[:, 0].T
    ccf[64:128, 0:NP] = b1f.reshape(NP, 2, 64)[:, 1].T
    thr = np.full((128, 8), 1e30, np.float32)
    s_ = np.arange(NSLOT)
    thr[s_ % 128, s_ // 128] = -b2[slot_t, slot_m]
    ccf[:, NP:NP + 8] = thr
    ccf[:, NP + 8:NP + 8 + 128] = np.eye(128, dtype=np.float32)

    # bf16 consts [128, NCB]: blbr8 | brc8 | wvec8 (tiled x8 over bt)
    ccb = np.zeros((128, NCB), ml_dtypes.bfloat16)
    blbr = np.zeros((320,), np.float32)
    brc = np.zeros((320,), np.float32)
    pos = np.arange(L)[:, None]
    tt = np.arange(T)[None, :]
    blbr[(pos * 20 + tt).ravel()] = (bl.T - br.T).ravel()
    brc[(pos * 20 + tt).ravel()] = br.T.ravel()
    ccb[:, 0:2560] = _bf16(np.tile(blbr, 8))[None, :]
    ccb[:, 2560:5120] = _bf16(np.tile(brc, 8))[None, :]
    wvec = NCLS - np.arange(NCLS, dtype=np.float32)
    ccb[:, 5120:5200] = _bf16(np.tile(wvec, 8))[None, :]

    shared = {"wtf": wtf, "wtb": wtb, "ccf": ccf, "ccb": ccb}

    xts, xbs = [], []
    for c in range(NCORE):
        xt = np.ascontiguousarray(
            x[c * BLOC:(c + 1) * BLOC].T).reshape(2, 128, BLOC)
        xr = _tf32(xt)
        xl = _tf32(xt - xr)
        xts.append(xr)
        xbs.append(np.concatenate([_bf16(xl), _bf16(xr)], axis=0))
    return shared, xts, xbs


def _build(nc: bass.Bass):
    xr_d = nc.dram_tensor("xr", [2, 128, BLOC], F32, kind="ExternalInput").ap()
    xb_d = nc.dram_tensor("xb", [4, 128, BLOC], BF16,
                          kind="ExternalInput").ap()
    wtf_d = nc.dram_tensor("wtf", [NP, 128, 384], F32,
                           kind="ExternalInput").ap()
    wtb_d = nc.dram_tensor("wtb", [NP, 128, 512], BF16,
                           kind="ExternalInput").ap()
    ccf_d = nc.dram_tensor("ccf", [128, NCF], F32, kind="ExternalInput").ap()
    ccb_d = nc.dram_tensor("ccb", [128, NCB], BF16, kind="ExternalInput").ap()
    out_d = nc.dram_tensor("out", [BLOC], F32, kind="ExternalOutput").ap()

    with TileContext(nc) as tc:
        with tc.tile_pool(name="const", bufs=1) as cp:
            xrt = cp.tile([128, 2 * BLOC], F32R, name="xrt")
            xbt = cp.tile([128, 4 * BLOC], BF16, name="xbt")
            ccf = cp.tile([128, NCF], F32, name="ccf")
            ccb = cp.tile([128, NCB], BF16, name="ccb")

            def dma_xcc():
                for k in range(2):
                    for cs in range(4):
                        nc.sync.dma_start(
                            xrt[:, k * BLOC + cs * 256:
                                k * BLOC + (cs + 1) * 256],
                            xr_d[k].bitcast(F32R)[:, cs * 256:(cs + 1) * 256])
                nc.sync.dma_start(ccf[:], ccf_d)
                for k in range(4):
                    for cs in range(2):
                        nc.sync.dma_start(
                            xbt[:, k * BLOC + cs * 512:
                                k * BLOC + (cs + 1) * 512],
                            xb_d[k][:, cs * 512:(cs + 1) * 512])
                nc.sync.dma_start(ccb[:], ccb_d)
            b1c = ccf[:, 0:NP]
            thr = ccf[:, NP:NP + 8]
            ident = ccf[:, NP + 8:NP + 8 + 128]
            blbr8 = ccb[:, 0:2560]
            brc8 = ccb[:, 2560:5120]
            wvec8 = ccb[:, 5120:5200]
            s_all = cp.tile([128, NZ * BLOC], F32, name="s_all")
            out_sb = cp.tile([128, BLOC // 128], F32, name="out_sb")

            gend = [64, 128, 192, 256, 310]

            with tc.tile_pool(name="wpf", bufs=6) as wpf, \
                 tc.tile_pool(name="wpb", bufs=6) as wpb, \
                 tc.tile_pool(name="hp", bufs=5) as hp, \
                 tc.tile_pool(name="stp", bufs=1) as stp, \
                 tc.tile_pool(name="wkp", bufs=2) as wkp, \
                 tc.tile_pool(name="php", bufs=2, space="PSUM") as php, \
                 tc.tile_pool(name="pzp", bufs=2, space="PSUM") as pzp, \
                 tc.tile_pool(name="ptp", bufs=2, space="PSUM") as ptp:
                st_all = stp.tile([128, 8 * 640], BF16, name="st_all")
                pz = {}

                def walk():
                    # tree walk for all 8 bt tiles at once (bf16 mask
                    # algebra on DVE; every value is a small exact int).
                    ctx = nc.allow_low_precision(
                        reason="walk values are small exact integers")
                    ctx.__enter__()
                    st3 = st_all[:].rearrange("p (b s) -> p b s", s=640)
                    m1 = wkp.tile([128, 8 * 40], BF16, name="m1")
                    m13 = m1[:].rearrange("p (b x) -> p b x", x=40)
                    nc.vector.tensor_copy(m13[:, :, 0:20], st3[:, :, 0:20])
                    nc.vector.tensor_scalar(m13[:, :, 20:40],
                                            st3[:, :, 0:20],
                                            -1.0, 1.0, OP.mult, OP.add)
                    mprev = m1
                    for k in range(1, 4):
                        nq = 2 ** k
                        off = (nq - 1) * 20
                        mn = wkp.tile([128, 8 * nq * 40], BF16,
                                      name=f"m{k + 1}")
                        mn5 = mn[:].rearrange("p (b q x) -> p b q x",
                                              b=8, q=nq, x=40)
                        sv = st3[:, :, off:off + nq * 20].rearrange(
                            "p b (q t) -> p b q t", t=20)
                        mv = mprev[:].rearrange("p (b q t) -> p b q t",
                                                b=8, q=nq, t=20)
                        nc.vector.tensor_tensor(mn5[:, :, :, 0:20], mv, sv,
                                                OP.mult)
                        nc.vector.tensor_tensor(mn5[:, :, :, 20:40], mv,
                                                mn5[:, :, :, 0:20],
                                                OP.subtract)
                        mprev = mn
                    val = wkp.tile([128, 8 * 320], BF16, name="val")
                    val3 = val[:].rearrange("p (b s) -> p b s", s=320)
                    nc.vector.tensor_tensor(val3, st3[:, :, 300:620],
                                            blbr8[:].rearrange(
                                                "p (b s) -> p b s", s=320),
                                            OP.mult)
                    nc.vector.tensor_tensor(val[:], val[:], brc8, OP.add)
                    prod = wkp.tile([128, 8 * 320], BF16, name="prod")
                    nc.vector.tensor_tensor(prod[:], mprev[:], val[:],
                                            OP.mult)
                    tadd = wkp.tile([128, 8 * 8 * 20], BF16, name="tadd")
                    pr4 = prod[:].rearrange("p (b q t) -> p b q t",
                                            b=8, q=16, t=20)
                    ta4 = tadd[:].rearrange("p (b q t) -> p b q t",
                                            b=8, q=8, t=20)
                    nc.vector.tensor_tensor(ta4, pr4[:, :, 0:8, :],
                                            pr4[:, :, 8:16, :], OP.add)
                    nc.vector.tensor_tensor(ta4[:, :, 0:4, :],
                                            ta4[:, :, 0:4, :],
                                            ta4[:, :, 4:8, :], OP.add)
                    nc.vector.tensor_tensor(ta4[:, :, 0:2, :],
                                            ta4[:, :, 0:2, :],
                                            ta4[:, :, 2:4, :], OP.add)
                    pred = wkp.tile([128, 8 * 20], BF16, name="pred")
                    nc.vector.tensor_tensor(
                        pred[:].rearrange("p (b t) -> p b t", t=20),
                        ta4[:, :, 0, :], ta4[:, :, 1, :], OP.add)
                    pred3 = pred[:].rearrange("p (b t) -> p b t", t=20)
                    counts = wkp.tile([128, 8 * NCLS], BF16, name="counts")
                    cn3 = counts[:].rearrange("p (b c) -> p b c", c=NCLS)
                    eqt = wkp.tile([128, 8 * 20], BF16, name="eqt")
                    eq3 = eqt[:].rearrange("p (b t) -> p b t", t=20)
                    for cls in range(NCLS):
                        nc.vector.tensor_scalar(eq3, pred3, float(cls),
                                                None, OP.is_equal)
                        nc.vector.tensor_reduce(cn3[:, :, cls:cls + 1],
                                                eq3, axis=AX.X, op=OP.add)
                    cmax = wkp.tile([128, 8], BF16, name="cmax")
                    nc.vector.tensor_reduce(cmax[:], cn3, axis=AX.X,
                                            op=OP.max)
                    cmr = wkp.tile([128, 8 * NCLS], BF16, name="cmr")
                    cmr3 = cmr[:].rearrange("p (b c) -> p b c", c=NCLS)
                    cm3 = cmax[:].rearrange("p (b o) -> p b o", o=1)
                    for cls in range(NCLS):
                        nc.vector.tensor_copy(cmr3[:, :, cls:cls + 1], cm3)
                    pick = wkp.tile([128, 8 * NCLS], BF16, name="pick")
                    nc.vector.tensor_tensor(pick[:], counts[:], cmr[:],
                                            OP.is_equal)
                    nc.vector.tensor_tensor(pick[:], pick[:], wvec8,
                                            OP.mult)
                    mv_ = wkp.tile([128, 8], BF16, name="mv_")
                    nc.vector.tensor_reduce(
                        mv_[:], pick[:].rearrange("p (b c) -> p b c", c=NCLS),
                        axis=AX.X, op=OP.max)
                    nc.vector.tensor_scalar(out_sb[:], mv_[:],
                                            -1.0, float(NCLS), OP.mult,
                                            OP.add)
                    ctx.__exit__(None, None, None)

                hrhl = {}
                pe_prev = [None]

                def pe(inst):
                    # chain every PE instruction: pins the scheduler to
                    # emission order so non-self-loading matmuls always
                    # follow their weight load with nothing in between.
                    if ELIDE and pe_prev[0] is not None:
                        s = _bass_rust.InstructionNameOrderedSet()
                        s.add(pe_prev[0])
                        inst.ins.add_nosync_dependencies_from(s)
                    pe_prev[0] = inst.ins.name
                    return inst

                pre = {}
                for u0 in range(2):
                    wtf_t = wpf.tile([128, 384], F32R, name="wtf")
                    nc.sync.dma_start(wtf_t[:], wtf_d[u0].bitcast(F32R))
                    wtb_t = wpb.tile([128, 512], BF16, name="wtb")
                    nc.sync.dma_start(wtb_t[:], wtb_d[u0])
                    pre[u0] = (wtf_t, wtb_t)
                dma_xcc()

                def emit_m1(u):
                    # matmul1 for pair u + relu/bias acts (ScalarE) + hl
                    # (DVE).  bf16 cross passes share weight loads via
                    # explicit ldweights + non-self-loading matmuls.
                    if u in pre:
                        wtf_t, wtb_t = pre[u]
                    else:
                        wtf_t = wpf.tile([128, 384], F32R, name="wtf")
                        nc.sync.dma_start(wtf_t[:], wtf_d[u].bitcast(F32R))
                        wtb_t = wpb.tile([128, 512], BF16, name="wtb")
                        nc.sync.dma_start(wtb_t[:], wtb_d[u])
                    ph = php.tile([128, BLOC], F32)
                    for half in range(2):
                        w = wtf_t[:, half * 128:(half + 1) * 128]
                        for cs in range(2):
                            mm = pe(nc.tensor.matmul(
                                ph[:, cs * 512:(cs + 1) * 512], w,
                                xrt[:, half * BLOC + cs * 512:
                                    half * BLOC + (cs + 1) * 512],
                                start=(half == 0), stop=False))
                            if ELIDE and cs == 1:
                                mm.ins.ldweights = False
                    for bi in range(4):
                        w = wtb_t[:, bi * 128:(bi + 1) * 128]
                        for cs in range(2):
                            mm = pe(nc.tensor.matmul(
                                ph[:, cs * 512:(cs + 1) * 512], w,
                                xbt[:, bi * BLOC + cs * 512:
                                    bi * BLOC + (cs + 1) * 512],
                                start=False, stop=(bi == 3)))
                            if ELIDE and cs == 1:
                                mm.ins.ldweights = False
                    hr = hp.tile([128, BLOC], F32R, name="hr")
                    nc.scalar.activation(hr[:], ph[:], ACT_RELU,
                                         bias=b1c[:, u:u + 1], scale=1.0)
                    h = hp.tile([128, BLOC], F32, name="h")
                    nc.scalar.activation(h[:], ph[:], ACT_RELU,
                                         bias=b1c[:, u:u + 1], scale=1.0)
                    hl = hp.tile([128, BLOC], F32R, name="hl")
                    nc.vector.tensor_tensor(hl[:], h[:], hr[:], OP.subtract)
                    hrhl[u] = (wtf_t, hr, hl)

                def emit_z(u):
                    # z accumulation for pair u; the sign matrix loads once
                    # and is reused by the remaining three matmuls.
                    zt = u // 64
                    first = u % 64 == 0
                    last = u == gend[zt] - 1
                    wtf_t, hr, hl = hrhl.pop(u)
                    sg = wtf_t[:, 256:384]
                    if first:
                        pz[zt] = [pzp.tile([128, 512], F32,
                                           name=f"pz{zt}_{c}", tag="pz")
                                  for c in range(2)]
                    nmm = 0
                    for c in range(2):
                        cs = slice(c * 512, (c + 1) * 512)
                        for src in (hr, hl):
                            mm = pe(nc.tensor.matmul(
                                pz[zt][c][:], sg, src[:, cs],
                                start=(first and src is hr),
                                stop=(last and src is hl)))
                            if ELIDE and nmm:
                                mm.ins.ldweights = False
                            nmm += 1
                    if last:
                        for c in range(2):
                            dst = s_all[:, zt * BLOC + c * 512:
                                        zt * BLOC + (c + 1) * 512]
                            nc.vector.tensor_scalar(dst, pz[zt][c][:],
                                                    thr[:, zt:zt + 1], None,
                                                    OP.is_gt)
                        for bt in range(BLOC // 128):
                            pt = ptp.tile([128, 128], F32)
                            pe(nc.tensor.transpose(
                                pt[:],
                                s_all[:, zt * BLOC + bt * 128:
                                      zt * BLOC + (bt + 1) * 128],
                                ident))
                            nc.vector.tensor_copy(
                                st_all[:, bt * 640 + zt * 128:
                                       bt * 640 + (zt + 1) * 128], pt[:])
                        if zt == NZ - 1:
                            walk()

                # software pipeline: z(u) emitted after m1(u+2) so its
                # hr/hl inputs are ready when the PE reaches it in order.
                LAG = LAG_CFG
                for u in range(NP):
                    emit_m1(u)
                    if u >= LAG:
                        emit_z(u - LAG)
                for u in range(NP - LAG, NP):
                    emit_z(u)

            nc.sync.dma_start(out_d.rearrange("(b p) -> p b", p=128),
                              out_sb[:])
    return nc


_CACHE = {}


def kernel(x, W1, b1, W2, b2, feats, best_left, best_right) -> np.ndarray:
    shared, xts, xbs = _pack_inputs(x, W1, b1, W2, b2, feats,
                                    best_left, best_right)
    if "nc" not in _CACHE:
        nc = bacc.Bacc("TRN2", target_bir_lowering=False, debug=False,
                       num_devices=NCORE)
        _build(nc)
        nc.compile()
        _CACHE["nc"] = nc
    nc = _CACHE["nc"]
    in_maps = [dict(shared, xr=xts[c], xb=xbs[c]) for c in range(NCORE)]
    res = bass_utils.run_bass_kernel_spmd(nc, in_maps,
                                          core_ids=list(range(NCORE)))
    out = np.concatenate([res.results[c]["out"] for c in range(NCORE)])
    return out.astype(np.float32)


# revision 27
# speedup vs baseline: 1.0009x; 1.0009x over previous
"""DeepForest (nn_DeepForest_30356828848186) Trainium2 Bass kernel.

Strategy: data-parallel over batch across the 8 NeuronCores (B=8192 ->
1024 rows/core, forest parameters replicated) -- no collectives needed.

Per core:
  * 620 node slots (level-major: slot = (m-1)*20 + t), 310 pairs; each
    pair packs 2 nodes x 64 hidden units onto the 128 PE partitions.
  * matmul1 computes y = x @ W1dense per pair into PSUM [128, 1024],
    where W1dense is the per-node feature-scattered W1 with |W2| folded
    in (w*relu(y) = sign(w)*relu(|w|*y)).  Error-compensated scheme
    validated numerically on the exact seed-0 data (err std ~5e-7,
    0/8192 label mismatches):
      pass1: f32r  Wr  @ xr      (tf32 hi x hi)
      pass2: bf16  Wr_b @ xl_b   (x residual correction)
      pass3: bf16  Wl_b @ xr_b   (W residual correction)
  * relu+bias via ScalarE activation (per-partition bias), producing an
    f32r-rounded hr and fp32 h; hl = h - hr (f32r) on VectorE.
  * z = sum_h sign(W2)*relu(...) via sign-matrix matmuls accumulated
    per 64-pair group into PSUM [128 slots, 512 B]; z = sgn@hr + sgn@hl.
  * gate bits s = (z > -b2) via per-partition threshold compare; PE
    transposes per group (interleaved with the matmul stream), then the
    tree walk as bf16 mask algebra on DVE (2x 16-bit throughput).

Self-contained: hardcodes all shapes; only needs numpy + concourse.
"""
import sys

import numpy as np
import ml_dtypes

sys.path.insert(0, "/opt/trn_rl_repo")

import concourse.bass as bass  # noqa: E402
import concourse.bacc as bacc  # noqa: E402
import concourse.mybir as mybir  # noqa: E402
from concourse.tile import TileContext  # noqa: E402
from concourse import bass_utils  # noqa: E402
import bass_rust as _bass_rust  # noqa: E402

F32 = mybir.dt.float32
F32R = mybir.dt.float32r
BF16 = mybir.dt.bfloat16
AX = mybir.AxisListType
OP = mybir.AluOpType
ACT_RELU = mybir.ActivationFunctionType.Relu

T, M, D, B, F, H = 20, 31, 5, 8192, 256, 64
L = 16
ELIDE = True
LAG_CFG = 2
NSLOT = 620          # node slots, no padding
NP = NSLOT // 2      # 310 node pairs
NZ = 5               # z-tiles (slot groups of 128)
NCORE = 8
BLOC = B // NCORE    # 1024
NCLS = 10
NCF = NP + 8 + 128   # f32 consts cols: b1c, thr(pad 8), ident
NCB = 2560 + 2560 + 80  # bf16 consts cols: blbr8, brc8, wvec8 (x8 bt)


def _tf32(a: np.ndarray) -> np.ndarray:
    bits = np.asarray(a, np.float32).view(np.uint32) + np.uint32(0x1000)
    bits &= np.uint32(0xFFFFE000)
    return bits.view(np.float32)


def _bf16(a: np.ndarray) -> np.ndarray:
    return np.asarray(a, np.float32).astype(ml_dtypes.bfloat16)


def _pack_inputs(x, W1, b1, W2, b2, feats, best_left, best_right):
    x = np.asarray(x, np.float32)
    W1 = np.asarray(W1, np.float32)
    b1 = np.asarray(b1, np.float32)
    W2 = np.asarray(W2, np.float32)
    b2 = np.asarray(b2, np.float32)
    feats = np.asarray(feats)
    bl = np.asarray(best_left).astype(np.float32)
    br = np.asarray(best_right).astype(np.float32)

    absW2 = np.abs(W2)
    sgnW2 = np.sign(W2).astype(np.float32)

    slot_t = np.arange(NSLOT) % 20
    slot_m = np.arange(NSLOT) // 20  # node index m-1 in 0..30

    # per-slot dense folded weights [620, 256, 64]
    W1s = (W1[slot_t, slot_m] * absW2[slot_t, slot_m][:, None, :]
           ).astype(np.float32)                      # [620, 128, 64]
    fs = feats[slot_t, slot_m]                       # [620, 128]
    W1f = np.zeros((NSLOT, F, H), np.float32)
    W1f[np.arange(NSLOT)[:, None, None], fs[:, :, None],
        np.arange(H)[None, None, :]] = W1s
    Wr = _tf32(W1f)
    Wl = _tf32(W1f - Wr)
    Wr_b = _bf16(Wr)
    Wl_b = _bf16(Wl)
    b1f = (b1[slot_t, slot_m] * absW2[slot_t, slot_m]).astype(np.float32)
    sgs = sgnW2[slot_t, slot_m]                      # [620, 64]

    # wtf [NP, 128, 384] f32: war_h0 | war_h1 | sgn
    wtf = np.zeros((NP, 128, 384), np.float32)
    Wrp = Wr.reshape(NP, 2, F, H)
    wtf[:, :, 0:64] = Wrp[:, 0, 0:128]
    wtf[:, :, 64:128] = Wrp[:, 1, 0:128]
    wtf[:, :, 128:192] = Wrp[:, 0, 128:256]
    wtf[:, :, 192:256] = Wrp[:, 1, 128:256]
    u = np.arange(NP)
    ca = (2 * u) % 128
    cb = (2 * u + 1) % 128
    wtf[u[:, None], np.arange(64)[None, :], (256 + ca)[:, None]] = \
        sgs.reshape(NP, 2, 64)[:, 0]
    wtf[u[:, None], (64 + np.arange(64))[None, :], (256 + cb)[:, None]] = \
        sgs.reshape(NP, 2, 64)[:, 1]

    # wtb [NP, 128, 512] bf16: wrb_h0 | wrb_h1 | wlb_h0 | wlb_h1
    wtb = np.zeros((NP, 128, 512), ml_dtypes.bfloat16)
    Wrbp = Wr_b.reshape(NP, 2, F, H)
    Wlbp = Wl_b.reshape(NP, 2, F, H)
    wtb[:, :, 0:64] = Wrbp[:, 0, 0:128]
    wtb[:, :, 64:128] = Wrbp[:, 1, 0:128]
    wtb[:, :, 128:192] = Wrbp[:, 0, 128:256]
    wtb[:, :, 192:256] = Wrbp[:, 1, 128:256]
    wtb[:, :, 256:320] = Wlbp[:, 0, 0:128]
    wtb[:, :, 320:384] = Wlbp[:, 1, 0:128]
    wtb[:, :, 384:448] = Wlbp[:, 0, 128:256]
    wtb[:, :, 448:512] = Wlbp[:, 1, 128:256]

    # f32 consts [128, NCF]: b1c | thr | ident
    ccf = np.zeros((128, NCF), np.float32)
    ccf[0:64, 0:NP] = b1f.reshape(NP, 2, 64)[:, 0].T
    ccf[64:128, 0:NP] = b1f.reshape(NP, 2, 64)[:, 1].T
    thr = np.full((128, 8), 1e30, np.float32)
    s_ = np.arange(NSLOT)
    thr[s_ % 128, s_ // 128] = -b2[slot_t, slot_m]
    ccf[:, NP:NP + 8] = thr
    ccf[:, NP + 8:NP + 8 + 128] = np.eye(128, dtype=np.float32)

    # bf16 consts [128, NCB]: blbr8 | brc8 | wvec8 (tiled x8 over bt)
    ccb = np.zeros((128, NCB), ml_dtypes.bfloat16)
    blbr = np.zeros((320,), np.float32)
    brc = np.zeros((320,), np.float32)
    pos = np.arange(L)[:, None]
    tt = np.arange(T)[None, :]
    blbr[(pos * 20 + tt).ravel()] = (bl.T - br.T).ravel()
    brc[(pos * 20 + tt).ravel()] = br.T.ravel()
    ccb[:, 0:2560] = _bf16(np.tile(blbr, 8))[None, :]
    ccb[:, 2560:5120] = _bf16(np.tile(brc, 8))[None, :]
    wvec = NCLS - np.arange(NCLS, dtype=np.float32)
    ccb[:, 5120:5200] = _bf16(np.tile(wvec, 8))[None, :]

    shared = {"wtf": wtf, "wtb": wtb, "ccf": ccf, "ccb": ccb}

    xts, xbs = [], []
    for c in range(NCORE):
        xt = np.ascontiguousarray(
            x[c * BLOC:(c + 1) * BLOC].T).reshape(2, 128, BLOC)
        xr = _tf32(xt)
        xl = _tf32(xt - xr)
        xts.append(xr)
        xbs.append(np.concatenate([_bf16(xl), _bf16(xr)], axis=0))
    return shared, xts, xbs


def _build(nc: bass.Bass):
    xr_d = nc.dram_tensor("xr", [2, 128, BLOC], F32, kind="ExternalInput").ap()
    xb_d = nc.dram_tensor("xb", [4, 128, BLOC], BF16,
                          kind="ExternalInput").ap()
    wtf_d = nc.dram_tensor("wtf", [NP, 128, 384], F32,
                           kind="ExternalInput").ap()
    wtb_d = nc.dram_tensor("wtb", [NP, 128, 512], BF16,
                           kind="ExternalInput").ap()
    ccf_d = nc.dram_tensor("ccf", [128, NCF], F32, kind="ExternalInput").ap()
    ccb_d = nc.dram_tensor("ccb", [128, NCB], BF16, kind="ExternalInput").ap()
    out_d = nc.dram_tensor("out", [BLOC], F32, kind="ExternalOutput").ap()

    with TileContext(nc) as tc:
        with tc.tile_pool(name="const", bufs=1) as cp:
            xrt = cp.tile([128, 2 * BLOC], F32R, name="xrt")
            xbt = cp.tile([128, 4 * BLOC], BF16, name="xbt")
            ccf = cp.tile([128, NCF], F32, name="ccf")
            ccb = cp.tile([128, NCB], BF16, name="ccb")

            def dma_xcc():
                for k in range(2):
                    for cs in range(2):
                        nc.sync.dma_start(
                            xrt[:, k * BLOC + cs * 512:
                                k * BLOC + (cs + 1) * 512],
                            xr_d[k].bitcast(F32R)[:, cs * 512:(cs + 1) * 512])
                nc.sync.dma_start(ccf[:], ccf_d)
                for k in range(4):
                    for cs in range(2):
                        nc.sync.dma_start(
                            xbt[:, k * BLOC + cs * 512:
                                k * BLOC + (cs + 1) * 512],
                            xb_d[k][:, cs * 512:(cs + 1) * 512])
                nc.sync.dma_start(ccb[:], ccb_d)
            b1c = ccf[:, 0:NP]
            thr = ccf[:, NP:NP + 8]
            ident = ccf[:, NP + 8:NP + 8 + 128]
            blbr8 = ccb[:, 0:2560]
            brc8 = ccb[:, 2560:5120]
            wvec8 = ccb[:, 5120:5200]
            s_all = cp.tile([128, NZ * BLOC], F32, name="s_all")
            out_sb = cp.tile([128, BLOC // 128], F32, name="out_sb")

            gend = [64, 128, 192, 256, 310]

            with tc.tile_pool(name="wpf", bufs=6) as wpf, \
                 tc.tile_pool(name="wpb", bufs=6) as wpb, \
                 tc.tile_pool(name="hp", bufs=5) as hp, \
                 tc.tile_pool(name="stp", bufs=1) as stp, \
                 tc.tile_pool(name="wkp", bufs=2) as wkp, \
                 tc.tile_pool(name="php", bufs=2, space="PSUM") as php, \
                 tc.tile_pool(name="pzp", bufs=2, space="PSUM") as pzp, \
                 tc.tile_pool(name="ptp", bufs=2, space="PSUM") as ptp:
                st_all = stp.tile([128, 8 * 640], BF16, name="st_all")
                pz = {}

                def walk():
                    # tree walk for all 8 bt tiles at once (bf16 mask
                    # algebra on DVE; every value is a small exact int).
                    ctx = nc.allow_low_precision(
                        reason="walk values are small exact integers")
                    ctx.__enter__()
                    st3 = st_all[:].rearrange("p (b s) -> p b s", s=640)
                    m1 = wkp.tile([128, 8 * 40], BF16, name="m1")
                    m13 = m1[:].rearrange("p (b x) -> p b x", x=40)
                    nc.vector.tensor_copy(m13[:, :, 0:20], st3[:, :, 0:20])
                    nc.vector.tensor_scalar(m13[:, :, 20:40],
                                            st3[:, :, 0:20],
                                            -1.0, 1.0, OP.mult, OP.add)
                    mprev = m1
                    for k in range(1, 4):
                        nq = 2 ** k
                        off = (nq - 1) * 20
                        mn = wkp.tile([128, 8 * nq * 40], BF16,
                                      name=f"m{k + 1}")
                        mn5 = mn[:].rearrange("p (b q x) -> p b q x",
                                              b=8, q=nq, x=40)
                        sv = st3[:, :, off:off + nq * 20].rearrange(
                            "p b (q t) -> p b q t", t=20)
                        mv = mprev[:].rearrange("p (b q t) -> p b q t",
                                                b=8, q=nq, t=20)
                        nc.vector.tensor_tensor(mn5[:, :, :, 0:20], mv, sv,
                                                OP.mult)
                        nc.vector.tensor_tensor(mn5[:, :, :, 20:40], mv,
                                                mn5[:, :, :, 0:20],
                                                OP.subtract)
                        mprev = mn
                    val = wkp.tile([128, 8 * 320], BF16, name="val")
                    val3 = val[:].rearrange("p (b s) -> p b s", s=320)
                    nc.vector.tensor_tensor(val3, st3[:, :, 300:620],
                                            blbr8[:].rearrange(
                                                "p (b s) -> p b s", s=320),
                                            OP.mult)
                    nc.vector.tensor_tensor(val[:], val[:], brc8, OP.add)
                    prod = wkp.tile([128, 8 * 320], BF16, name="prod")
                    nc.vector.tensor_tensor(prod[:], mprev[:], val[:],
                                            OP.mult)
                    tadd = wkp.tile([128, 8 * 8 * 20], BF16, name="tadd")
                    pr4 = prod[:].rearrange("p (b q t) -> p b q t",
                                            b=8, q=16, t=20)
                    ta4 = tadd[:].rearrange("p (b q t) -> p b q t",
                                            b=8, q=8, t=20)
                    nc.vector.tensor_tensor(ta4, pr4[:, :, 0:8, :],
                                            pr4[:, :, 8:16, :], OP.add)
                    nc.vector.tensor_tensor(ta4[:, :, 0:4, :],
                                            ta4[:, :, 0:4, :],
                                            ta4[:, :, 4:8, :], OP.add)
                    nc.vector.tensor_tensor(ta4[:, :, 0:2, :],
                                            ta4[:, :, 0:2, :],
                                            ta4[:, :, 2:4, :], OP.add)
                    pred = wkp.tile([128, 8 * 20], BF16, name="pred")
                    nc.vector.tensor_tensor(
                        pred[:].rearrange("p (b t) -> p b t", t=20),
                        ta4[:, :, 0, :], ta4[:, :, 1, :], OP.add)
                    pred3 = pred[:].rearrange("p (b t) -> p b t", t=20)
                    counts = wkp.tile([128, 8 * NCLS], BF16, name="counts")
                    cn3 = counts[:].rearrange("p (b c) -> p b c", c=NCLS)
                    eqt = wkp.tile([128, 8 * 20], BF16, name="eqt")
                    eq3 = eqt[:].rearrange("p (b t) -> p b t", t=20)
                    for cls in range(NCLS):
                        nc.vector.tensor_scalar(eq3, pred3, float(cls),
                                                None, OP.is_equal)
                        nc.vector.tensor_reduce(cn3[:, :, cls:cls + 1],
                                                eq3, axis=AX.X, op=OP.add)
                    cmax = wkp.tile([128, 8], BF16, name="cmax")
                    nc.vector.tensor_reduce(cmax[:], cn3, axis=AX.X,
                                            op=OP.max)
                    cmr = wkp.tile([128, 8 * NCLS], BF16, name="cmr")
                    cmr3 = cmr[:].rearrange("p (b c) -> p b c", c=NCLS)
                    cm3 = cmax[:].rearrange("p (b o) -> p b o", o=1)
                    for cls in range(NCLS):
                        nc.vector.tensor_copy(cmr3[:, :, cls:cls + 1], cm3)
                    pick = wkp.tile([128, 8 * NCLS], BF16, name="pick")
                    nc.vector.tensor_tensor(pick[:], counts[:], cmr[:],
                                            OP.is_equal)
                    nc.vector.tensor_tensor(pick[:], pick[:], wvec8,
                                            OP.mult)
                    mv_ = wkp.tile([128, 8], BF16, name="mv_")
                    nc.vector.tensor_reduce(
                        mv_[:], pick[:].rearrange("p (b c) -> p b c", c=NCLS),
                        axis=AX.X, op=OP.max)
                    nc.vector.tensor_scalar(out_sb[:], mv_[:],
                                            -1.0, float(NCLS), OP.mult,
                                            OP.add)
                    ctx.__exit__(None, None, None)

                hrhl = {}
                pe_prev = [None]

                def pe(inst):
                    # chain every PE instruction: pins the scheduler to
                    # emission order so non-self-loading matmuls always
                    # follow their weight load with nothing in between.
                    if ELIDE and pe_prev[0] is not None:
                        s = _bass_rust.InstructionNameOrderedSet()
                        s.add(pe_prev[0])
                        inst.ins.add_nosync_dependencies_from(s)
                    pe_prev[0] = inst.ins.name
                    return inst

                pre = {}
                for u0 in range(2):
                    wtf_t = wpf.tile([128, 384], F32R, name="wtf")
                    nc.sync.dma_start(wtf_t[:], wtf_d[u0].bitcast(F32R))
                    wtb_t = wpb.tile([128, 512], BF16, name="wtb")
                    nc.sync.dma_start(wtb_t[:], wtb_d[u0])
                    pre[u0] = (wtf_t, wtb_t)
                dma_xcc()

                def emit_m1(u):
                    # matmul1 for pair u + relu/bias acts (ScalarE) + hl
                    # (DVE).  bf16 cross passes share weight loads via
                    # explicit ldweights + non-self-loading matmuls.
                    if u in pre:
                        wtf_t, wtb_t = pre[u]
                    else:
                        wtf_t = wpf.tile([128, 384], F32R, name="wtf")
                        nc.sync.dma_start(wtf_t[:], wtf_d[u].bitcast(F32R))
                        wtb_t = wpb.tile([128, 512], BF16, name="wtb")
                        nc.sync.dma_start(wtb_t[:], wtb_d[u])
                    ph = php.tile([128, BLOC], F32)
                    for half in range(2):
                        w = wtf_t[:, half * 128:(half + 1) * 128]
                        for cs in range(2):
                            mm = pe(nc.tensor.matmul(
                                ph[:, cs * 512:(cs + 1) * 512], w,
                                xrt[:, half * BLOC + cs * 512:
                                    half * BLOC + (cs + 1) * 512],
                                start=(half == 0), stop=False))
                            if ELIDE and cs == 1:
                                mm.ins.ldweights = False
                    for bi in range(4):
                        w = wtb_t[:, bi * 128:(bi + 1) * 128]
                        for cs in range(2):
                            mm = pe(nc.tensor.matmul(
                                ph[:, cs * 512:(cs + 1) * 512], w,
                                xbt[:, bi * BLOC + cs * 512:
                                    bi * BLOC + (cs + 1) * 512],
                                start=False, stop=(bi == 3)))
                            if ELIDE and cs == 1:
                                mm.ins.ldweights = False
                    hr = hp.tile([128, BLOC], F32R, name="hr")
                    nc.scalar.activation(hr[:], ph[:], ACT_RELU,
                                         bias=b1c[:, u:u + 1], scale=1.0)
                    h = hp.tile([128, BLOC], F32, name="h")
                    nc.scalar.activation(h[:], ph[:], ACT_RELU,
                                         bias=b1c[:, u:u + 1], scale=1.0)
                    hl = hp.tile([128, BLOC], F32R, name="hl")
                    nc.vector.tensor_tensor(hl[:], h[:], hr[:], OP.subtract)
                    hrhl[u] = (wtf_t, hr, hl)

                def emit_z(u):
                    # z accumulation for pair u; the sign matrix loads once
                    # and is reused by the remaining three matmuls.
                    zt = u // 64
                    first = u % 64 == 0
                    last = u == gend[zt] - 1
                    wtf_t, hr, hl = hrhl.pop(u)
                    sg = wtf_t[:, 256:384]
                    if first:
                        pz[zt] = [pzp.tile([128, 512], F32,
                                           name=f"pz{zt}_{c}", tag="pz")
                                  for c in range(2)]
                    nmm = 0
                    for c in range(2):
                        cs = slice(c * 512, (c + 1) * 512)
                        for src in (hr, hl):
                            mm = pe(nc.tensor.matmul(
                                pz[zt][c][:], sg, src[:, cs],
                                start=(first and src is hr),
                                stop=(last and src is hl)))
                            if ELIDE and nmm:
                                mm.ins.ldweights = False
                            nmm += 1
                    if last:
                        for c in range(2):
                            dst = s_all[:, zt * BLOC + c * 512:
                                        zt * BLOC + (c + 1) * 512]
                            nc.vector.tensor_scalar(dst, pz[zt][c][:],
                                                    thr[:, zt:zt + 1], None,
                                                    OP.is_gt)
                        for bt in range(BLOC // 128):
                            pt = ptp.tile([128, 128], F32)
                            pe(nc.tensor.transpose(
                                pt[:],
                                s_all[:, zt * BLOC + bt * 128:
                                      zt * BLOC + (bt + 1) * 128],
                                ident))
                            nc.vector.tensor_copy(
                                st_all[:, bt * 640 + zt * 128:
                                       bt * 640 + (zt + 1) * 128], pt[:])
                        if zt == NZ - 1:
                            walk()

                # software pipeline: z(u) emitted after m1(u+2) so its
                # hr/hl inputs are ready when the PE reaches it in order.
                LAG = LAG_CFG
                for u in range(NP):
                    emit_m1(u)
                    if u >= LAG:
                        emit_z(u - LAG)
                for u in range(NP - LAG, NP):
                    emit_z(u)

            nc.sync.dma_start(out_d.rearrange("(b p) -> p b", p=128),
                              out_sb[:])
    return nc


_CACHE = {}


def kernel(x, W1, b1, W2, b2, feats, best_left, best_right) -> np.ndarray:
    shared, xts, xbs = _pack_inputs(x, W1, b1, W2, b2, feats,
                                    best_left, best_right)
    if "nc" not in _CACHE:
        nc = bacc.Bacc("TRN2", target_bir_lowering=False, debug=False,
                       num_devices=NCORE)
        _build(nc)
        nc.compile()
        _CACHE["nc"] = nc
    nc = _CACHE["nc"]
    in_maps = [dict(shared, xr=xts[c], xb=xbs[c]) for c in range(NCORE)]
    res = bass_utils.run_bass_kernel_spmd(nc, in_maps,
                                          core_ids=list(range(NCORE)))
    out = np.concatenate([res.results[c]["out"] for c in range(NCORE)])
    return out.astype(np.float32)
